# revision 3
# baseline (speedup 1.0000x reference)
"""DiffPathRenderer Trainium2 kernel, v4.

Layout B: partition = (stroke, segment), free = pixels (j-major).
Per core: 32 strokes x 16 segs = 512 seg-partitions = 4 blocks of
[128 = 8 strokes x 16 segs]. Pixels f = j*128 + i, chunked 512 (4 j-cols).

Per (block, chunk):
  t1  = A*j + B*i + C            PE matmul K=3 (host coefs)   -> PSUM
  t   = clip(t1, 0, 1)           DVE tensor_scalar            -> SBUF
  mx  = (j - vx) - wvx*t         PE: affine MM + diag MM accum-> PSUM
  my  = (i - vy) - wvy*t         PE: affine MM + diag MM accum-> PSUM
  sx  = mx^2                     ACT Square                   -> SBUF
  sy  = my^2                     ACT Square                   -> SBUF
  d2  = sx + sy (bf16)           GPSIMD add                   -> SBUF
  dT  = transpose(d2)            PE transpose (4x 128x128)    -> PSUM
  dmin= min over 16 segs         DVE tensor_reduce (X axis)   -> SBUF

Finals: one Sqrt + one Sigmoid over [128, 4096], single DMA out.
"""

import numpy as np

import concourse.bacc as bacc
import concourse.mybir as mybir
import concourse.tile as tile
from concourse.bass_utils import run_bass_kernel_spmd

F32 = mybir.dt.float32
BF16 = mybir.dt.bfloat16
N_CORES = 8
B_TOTAL = 256
B_CORE = B_TOTAL // N_CORES   # 32
NSEG = 16
NBLK = 4                      # blocks of 8 strokes x 16 segs
SPB = 8                       # strokes per block
P = 128
NPIX = P * P                  # 16384
CH = 512                      # pixels per chunk (4 j-cols)
NCH = NPIX // CH              # 32 chunks

_cached = {}


def _build_bass():
    nc = bacc.Bacc(None)
    basis = nc.declare_dram_parameter("basis", [3, NPIX], F32, isOutput=False)
    lhs = nc.declare_dram_parameter("lhs", [3, NBLK * 3 * P], F32, isOutput=False)
    diag = nc.declare_dram_parameter("diag", [P, NBLK * 2 * P], F32, isOutput=False)
    fin = nc.declare_dram_parameter("fin", [P, 2], F32, isOutput=False)
    identb = nc.declare_dram_parameter("identb", [P, P], BF16, isOutput=False)
    out = nc.declare_dram_parameter("out", [B_CORE, P, P], F32, isOutput=True)

    AL = mybir.AluOpType
    AF = mybir.ActivationFunctionType
    AX = mybir.AxisListType

    with tile.TileContext(nc) as tc:
        with tc.tile_pool(name="main", bufs=1) as cpool, \
             tc.tile_pool(name="work", bufs=3) as wp, \
             tc.tile_pool(name="psum", bufs=2, space="PSUM") as pp:
            basis_t = cpool.tile([3, NPIX], F32, name="basis_t")
            nc.gpsimd.dma_start(out=basis_t[:], in_=basis[:])
            lhs_t = cpool.tile([3, NBLK * 3 * P], F32, name="lhs_t")
            nc.gpsimd.dma_start(out=lhs_t[:], in_=lhs[:])
            diag_t = cpool.tile([P, NBLK * 2 * P], F32, name="diag_t")
            nc.gpsimd.dma_start(out=diag_t[:], in_=diag[:])
            fin_t = cpool.tile([P, 2], F32, name="fin_t")
            nc.gpsimd.dma_start(out=fin_t[:], in_=fin[:])
            id_t = cpool.tile([P, P], BF16, name="id_t")
            nc.gpsimd.dma_start(out=id_t[:], in_=identb[:])

            dmin_t = cpool.tile([P, NBLK * SPB * P], F32, name="dmin_t")
            qd_t = cpool.tile([P, NBLK * SPB * P], F32, name="qd_t")

            def lhsv(b, kind):
                off = (b * 3 + kind) * P
                return lhs_t[0:3, off:off + P]

            def diagv(b, kind):
                off = (b * 2 + kind) * P
                return diag_t[:, off:off + P]

            for b in range(NBLK):
                dmin_b = dmin_t[:, b * SPB * P:(b + 1) * SPB * P] \
                    .rearrange("p (s j) -> p j s", s=SPB)
                for c in range(NCH):
                    bas = basis_t[0:3, c * CH:(c + 1) * CH]
                    pt1 = pp.tile([P, CH], F32, tag="t1", name="pt1")
                    nc.tensor.matmul(pt1[:], lhsv(b, 0), bas,
                                     start=True, stop=True)
                    t = wp.tile([P, CH], F32, tag="t", name="t")
                    nc.vector.tensor_scalar(t[:], pt1[:], 0.0, 1.0,
                                            AL.max, AL.min)

                    pmx = pp.tile([P, CH], F32, tag="mx", name="pmx")
                    nc.tensor.matmul(pmx[:], lhsv(b, 1), bas,
                                     start=True, stop=False)
                    nc.tensor.matmul(pmx[:], diagv(b, 0), t[:],
                                     start=False, stop=True)
                    pmy = pp.tile([P, CH], F32, tag="my", name="pmy")
                    nc.tensor.matmul(pmy[:], lhsv(b, 2), bas,
                                     start=True, stop=False)
                    nc.tensor.matmul(pmy[:], diagv(b, 1), t[:],
                                     start=False, stop=True)

                    sx = wp.tile([P, CH], F32, tag="sx", name="sx")
                    nc.scalar.activation(sx[:], pmx[:], AF.Square)
                    sy = wp.tile([P, CH], F32, tag="sy", name="sy")
                    nc.scalar.activation(sy[:], pmy[:], AF.Square)
                    d2 = wp.tile([P, CH], BF16, tag="d2", name="d2")
                    nc.gpsimd.tensor_tensor(d2[:], sx[:], sy[:], AL.add)

                    pdT = pp.tile([P, CH], BF16, tag="dT", name="pdT")
                    for k in range(4):
                        nc.tensor.transpose(pdT[:, k * P:(k + 1) * P],
                                            d2[:, k * P:(k + 1) * P], id_t[:])
                    rin = pdT[:].rearrange("p (k s g) -> p k s g", k=4, s=SPB)
                    rout = dmin_b[:, 4 * c:4 * c + 4, :]
                    nc.vector.tensor_reduce(rout, rin, axis=AX.X, op=AL.min)

            # finals: one Sqrt, one Sigmoid (table loads batched), one DMA
            nc.scalar.activation(qd_t[:], dmin_t[:], AF.Sqrt)
            nc.scalar.activation(dmin_t[:], qd_t[:], AF.Sigmoid,
                                 scale=fin_t[:, 0:1], bias=fin_t[:, 1:2])
            ov = out[:].rearrange("k i j -> i k j")
            sv = dmin_t[:].rearrange("i (k j) -> i k j", k=B_CORE)
            nc.sync.dma_start(out=ov, in_=sv)
    nc.finalize()
    return nc


def _host_coefs(traj, thickness):
    import ml_dtypes
    traj = np.asarray(traj, dtype=np.float32)
    T = traj * np.float32(128.0)
    v = T[:, :-1]                                 # (256, 16, 2)
    w = T[:, 1:]
    wv = w - v
    wvx, wvy = wv[..., 0], wv[..., 1]
    vx, vy = v[..., 0], v[..., 1]
    e2 = wvx * wvx + wvy * wvy + np.float32(1e-5)
    inv = np.float32(1.0) / e2
    A = wvx * inv
    Bc = wvy * inv
    C = -(vx * wvx + vy * wvy) * inv

    # basis: f = j*128 + i
    f = np.arange(NPIX, dtype=np.float32)
    basis = np.stack([f // P, f % P, np.ones(NPIX, np.float32)])  # (3, NPIX)

    thick = np.float32(np.asarray(thickness))
    r = thick / np.float32(2.0)
    fin = np.zeros((P, 2), np.float32)
    fin[:, 0] = -np.float32(70.0) / r
    fin[:, 1] = np.float32(7.0)

    identb = np.eye(P, dtype=ml_dtypes.bfloat16)

    in_maps = []
    for core in range(N_CORES):
        s0 = core * B_CORE
        lhs = np.zeros((3, NBLK * 3 * P), np.float32)
        diag = np.zeros((P, NBLK * 2 * P), np.float32)
        for b in range(NBLK):
            ks = s0 + b * SPB          # first stroke of block
            # partition p = s*16+g  ->  stroke ks+s, seg g
            Ap = A[ks:ks + SPB].reshape(P)
            Bp = Bc[ks:ks + SPB].reshape(P)
            Cp = C[ks:ks + SPB].reshape(P)
            vxp = vx[ks:ks + SPB].reshape(P)
            vyp = vy[ks:ks + SPB].reshape(P)
            wvxp = wvx[ks:ks + SPB].reshape(P)
            wvyp = wvy[ks:ks + SPB].reshape(P)
            o = b * 3 * P
            lhs[0, o:o + P] = Ap
            lhs[1, o:o + P] = Bp
            lhs[2, o:o + P] = Cp
            lhs[0, o + P:o + 2 * P] = 1.0
            lhs[2, o + P:o + 2 * P] = -vxp
            lhs[1, o + 2 * P:o + 3 * P] = 1.0
            lhs[2, o + 2 * P:o + 3 * P] = -vyp
            od = b * 2 * P
            diag[:, od:od + P][np.arange(P), np.arange(P)] = -wvxp
            diag[:, od + P:od + 2 * P][np.arange(P), np.arange(P)] = -wvyp
        in_maps.append({"basis": basis, "lhs": lhs, "diag": diag,
                        "fin": fin, "identb": identb})
    return in_maps


def kernel(traj, thickness):
    if "nc" not in _cached:
        _cached["nc"] = _build_bass()
    in_maps = _host_coefs(traj, thickness)
    res = run_bass_kernel_spmd(_cached["nc"], in_maps, list(range(N_CORES)))
    return np.concatenate([res.results[c]["out"] for c in range(N_CORES)], axis=0)
